# revision 32
# baseline (speedup 1.0000x reference)
"""2-hop GCN (gnn_message_passing) on 8 trn2 NeuronCores via Bass.

Algorithm (reference): h = A_hat^2 x;  out = log_softmax(h @ W + b)
  A_hat = D^-1/2 (A + I) D^-1/2  with D = in-degree+1 of (A + I).

Restructured for the device:
  Y  = (x @ W)                     [N, C]   (matmul first: propagation is linear)
  Z0 = dinv * Y                    (node-wise scale)
  S1 = sum_{edges} Z0[src] + Z0[t] (self-loop added at drain, not gathered)
  Z2 = dinv^2 * S1
  S3 = same with Z2
  out = log_softmax(dinv * S3 + b)

Sharding: nodes (and edges by target) across 8 cores; full Z exchanged
between hops via AllGather collectives. Propagation data in fp16 (PSUM
accumulation exact fp32); head/tail in fp32.

Per-hop device pattern per core:
  - dma_gather (SWDGE) pulls Z[src] rows (fp16, 80B payload, 256B-strided
    source) for edge chunks of 128, batched 24 chunks (3072 idx) per
    instruction with single_packet=False (with single_packet=True the
    per-engine descriptor stream must stay under ~64 descs = 1024 idxs,
    or the device dies).
  - Edges are pre-sorted by (src-half, target window of 128); per target
    window the one-hot S tiles are built in one batched DVE is_equal per
    (half, window) run, then PE matmuls S^T @ msgs accumulate the window's
    PSUM bank.
  - Windows drain with (bank + Z_local) * dinv-scale fused on DVE.
"""
import math

import numpy as np

import concourse.bacc as bacc
import concourse.bass as bass
import concourse.mybir as mybir
import concourse.tile as tile
from concourse.bass_utils import run_bass_kernel_spmd
from concourse.masks import make_identity

F32 = mybir.dt.float32
F16 = mybir.dt.float16
I16 = mybir.dt.int16

LAST_RESULTS = None  # test harness peeks at this after kernel(..., _trace=True)

N_SWDGE_QUEUES = 1


class Cfg:
    def __init__(self, N=50000, F=100, C=40, CORES=8, W_WIN=128, BATCH_SLOTS=24,
                 ZPAD=128):
        self.N, self.F, self.C, self.CORES = N, F, C, CORES
        assert N % CORES == 0
        self.NPC = N // CORES
        assert N % 2 == 0
        self.HALF = N // 2
        assert self.HALF <= 32768, "gather idx must fit int16"
        self.W_WIN = W_WIN                      # targets per PSUM window
        self.N_WIN = math.ceil(self.NPC / W_WIN)
        self.BATCH_SLOTS = BATCH_SLOTS          # chunks per gather instruction
        self.BIDX = BATCH_SLOTS * 128           # idxs per gather instruction
        self.ZPAD = ZPAD                        # fp16 row stride of Z (256B)
        assert (ZPAD * 2) % 256 == 0
        self.NT_HEAD = math.ceil(self.NPC / 128)


def _wrap_idx(flat):
    """idx position i -> [i%16, i//16], replicated to 128 partitions."""
    n = flat.shape[0]
    assert n % 16 == 0
    w = flat.reshape(n // 16, 16).T  # [16, n/16]
    return np.tile(w, (2, 1))        # [32, n/16] (queue-0 pair reads 32 ch)


def preprocess(edge_index, cfg):
    """Build the shared compile-time schedule and per-core device arrays."""
    N, NPC, W_WIN, N_WIN = cfg.N, cfg.NPC, cfg.W_WIN, cfg.N_WIN
    CH = 128
    row = np.asarray(edge_index[0], dtype=np.int64)
    col = np.asarray(edge_index[1], dtype=np.int64)

    deg = np.bincount(col, minlength=N).astype(np.float64) + 1.0
    dinv = 1.0 / np.sqrt(deg)

    cores = []
    counts = np.zeros((cfg.CORES, 2, N_WIN), np.int64)
    for c in range(cfg.CORES):
        lo = c * NPC
        m = (col >= lo) & (col < lo + NPC)
        s = row[m]
        d = col[m] - lo
        h = (s >= cfg.HALF).astype(np.int64)
        w = d // W_WIN
        order = np.lexsort((d, w, h))
        s, d, h, w = s[order], d[order], h[order], w[order]
        cores.append((s, d, h, w))
        for half in (0, 1):
            counts[c, half] = np.bincount(w[h == half], minlength=N_WIN)

    cpg = -(-counts // CH)      # ceil
    cpg = cpg.max(axis=0)       # [2, N_WIN] chunks per (half, window)

    slots = []                  # list of (half, win)
    for half in (0, 1):
        for w_ in range(N_WIN):
            slots += [(half, w_)] * int(cpg[half, w_])
        pad = (-len(slots)) % max(cfg.BATCH_SLOTS, 16)
        slots += [(half, N_WIN - 1)] * pad
    n_slots = len(slots)
    n_batches = n_slots // cfg.BATCH_SLOTS
    batch_half = [slots[b * cfg.BATCH_SLOTS][0] for b in range(n_batches)]
    for b in range(n_batches):
        assert all(s[0] == batch_half[b]
                   for s in slots[b * cfg.BATCH_SLOTS:(b + 1) * cfg.BATCH_SLOTS])

    # group -> its slot indices (gather order: half-major for batch purity).
    # Within a group, slot ids are a consecutive run.
    group_slots = {}
    for j, (h_, w_) in enumerate(slots):
        group_slots.setdefault((h_, w_), []).append(j)
    for sl in group_slots.values():
        assert sl == list(range(sl[0], sl[0] + len(sl)))

    # matmul order: window-major (one PSUM bank per window, <=8 in flight);
    # each window is up to two consecutive slot runs (half 0, half 1)
    win_runs = [[(sl[0], len(sl)) for sl in
                 (group_slots.get((0, w_)), group_slots.get((1, w_))) if sl]
                for w_ in range(N_WIN)]
    sched = dict(slots=slots, n_slots=n_slots, n_batches=n_batches,
                 batch_half=batch_half, win_runs=win_runs)

    per_core = []
    for c in range(cfg.CORES):
        s, d, h, w = cores[c]
        gidx = np.zeros(n_slots * CH, np.int16)
        dstw = np.full((CH, n_slots), 255.0, np.float16)
        for half in (0, 1):
            hm = h == half
            sh, dh, wh = s[hm], d[hm], w[hm]
            for w_ in range(N_WIN):
                wm = wh == w_
                se = sh[wm] - half * cfg.HALF
                de = dh[wm] - w_ * W_WIN
                sl = group_slots[(half, w_)]
                for k in range(se.shape[0]):
                    j = sl[k // CH]
                    lane = k % CH
                    gidx[j * CH + lane] = se[k]
                    dstw[lane, j] = de[k]
        gidx_w = np.concatenate(
            [_wrap_idx(gidx[b * cfg.BIDX:(b + 1) * cfg.BIDX])
             for b in range(n_batches)], axis=0)  # [n_batches*128, BIDX/16]

        lo = c * NPC
        dl = dinv[lo:lo + NPC]
        d1w = np.zeros((128, N_WIN), np.float32)
        for p in range(N_WIN):
            r = min(128, NPC - p * 128)
            d1w[:r, p] = dl[p * 128:p * 128 + r]
        per_core.append(dict(gidx=gidx_w, dstw=dstw,
                             d1w=d1w, d2w=d1w * d1w))
    return sched, per_core


def dma_gather_raw(nc, out_ap, in_ap, idxs_ap, num_idxs, elem_size, elem_step,
                   queue_num=0):
    """dma_gather without the blanket elem_size%256B assert (payload may be
    byte-granular; only the source row stride is 256B-encoded)."""
    eng = nc.gpsimd
    dts = mybir.dt.size(in_ap.dtype)
    stride_bytes = elem_step * dts
    assert stride_bytes % 256 == 0 and stride_bytes // 256 < 256
    assert in_ap.ap[0][0] == elem_step and in_ap.ap[-1][1] == elem_size
    assert idxs_ap.tensor.dtype == I16
    _in_ap = eng.lower_ap_dma(in_ap, for_custom_bir_dma=True)
    _idxs_ap = eng.lower_ap(idxs_ap)
    _out_ap = eng.lower_ap(out_ap)
    return eng.add_instruction(mybir.InstDMAGatherAnt(
        name=nc.get_next_instruction_name(),
        ins=[*_in_ap, _idxs_ap, eng.lower_val_access(eng.to_reg(num_idxs))],
        outs=[_out_ap],
        transpose=False, num_idxs=num_idxs, elem_size=elem_size,
        stride_bytes_256=stride_bytes // 256, gen_mode=0, single_packet=False,
        queue_num=queue_num, sbuf_tokens_per_rank=0, sbuf_free_dim_per_rank=0,
        sbuf_free_dim_pad_per_rank=0, sbuf_byte_offset=0))


def emit_hop(nc, tc, cfg, sched, pools, z_pad, dstw_sb, iota_sb, gidx_dram,
             out_assemble, hop_tag):
    """One propagation hop: gather + one-hot matmul segment-sum + drain.

    z_pad: DRAM [N, ZPAD] fp16 gather source.
    out_assemble: callback(w, src_psum_ap) emitting the drain for window w.
    """
    C, WW = cfg.C, cfg.W_WIN
    psum_pool, gpool, spool, ipool = (pools[k] for k in
                                      ("psum", "gather", "s", "idx"))
    # gather ALL batches up front (tiles stay resident; matmuls consume in
    # window order while later batches still stream in)
    msgs = []
    for b in range(sched["n_batches"]):
        h = sched["batch_half"][b]
        idx_sb = ipool.tile([32, cfg.BIDX // 16], I16, tag="idx")
        nc.sync.dma_start(
            out=idx_sb[:],
            in_=gidx_dram[b * 32:(b + 1) * 32, :])
        msg = gpool.tile([128, cfg.BATCH_SLOTS, C], F16, tag=f"msg{b}")
        src_view = z_pad[h * cfg.HALF:(h + 1) * cfg.HALF, 0:C]
        dma_gather_raw(nc, msg[:], src_view, idx_sb[:], cfg.BIDX,
                       elem_size=C, elem_step=cfg.ZPAD,
                       queue_num=b % N_SWDGE_QUEUES)
        msgs.append(msg)

    # one PSUM bank per window; pool rotation keeps <=8 windows in flight
    for w_ in range(cfg.N_WIN):
        bank = psum_pool.tile([128, 512], F32, tag="bank",
                              name=f"{hop_tag}_bank_w{w_}")
        runs = sched["win_runs"][w_]
        nrun = sum(ln for _, ln in runs)
        k = 0
        for j0, ln in runs:
            # batched one-hot build: S[:, i, :] one-hot of dstw col j0+i
            S = spool.tile([128, ln, WW], F16, tag="S")
            nc.vector.tensor_tensor(
                out=S[:],
                in0=iota_sb[:, :WW].unsqueeze(1).to_broadcast([128, ln, WW]),
                in1=dstw_sb[:, j0:j0 + ln].unsqueeze(2).to_broadcast(
                    [128, ln, WW]),
                op=mybir.AluOpType.is_equal)
            for i in range(ln):
                j = j0 + i
                b, q = j // cfg.BATCH_SLOTS, j % cfg.BATCH_SLOTS
                nc.tensor.matmul(
                    out=bank[0:WW, 0:C],
                    lhsT=S[:, i, :], rhs=msgs[b][:, q, :],
                    start=(k == 0),
                    stop=(k == nrun - 1))
                k += 1
        out_assemble(w_, bank[0:WW, 0:C])


def build_program(cfg, sched):
    nc = bacc.Bacc("TRN2", target_bir_lowering=False, debug=False,
                   num_devices=cfg.CORES, num_swdge_queues=N_SWDGE_QUEUES)
    N, F_, C, NPC, WW, NW = cfg.N, cfg.F, cfg.C, cfg.NPC, cfg.W_WIN, cfg.N_WIN

    xT = nc.dram_tensor("xT", [F_, NPC], F32, kind="ExternalInput")
    W = nc.dram_tensor("W", [F_, C], F32, kind="ExternalInput")
    bvec = nc.dram_tensor("bvec", [1, C], F32, kind="ExternalInput")
    d1w = nc.dram_tensor("d1w", [128, NW], F32, kind="ExternalInput")
    d2w = nc.dram_tensor("d2w", [128, NW], F32, kind="ExternalInput")
    gidx = nc.dram_tensor("gidx", [sched["n_batches"] * 32, cfg.BIDX // 16],
                          I16, kind="ExternalInput")
    dstw = nc.dram_tensor("dstw", [128, sched["n_slots"]], F16,
                          kind="ExternalInput")
    out = nc.dram_tensor("out", [NPC, C], F32, kind="ExternalOutput")

    # Collectives run on the 256B-padded row layout directly: the AllGather
    # output IS the gather source, killing the 50k-descriptor pad copy that
    # otherwise sits serially on the critical path (~190us per hop).
    cc1_in = nc.dram_tensor("cc1_in", [NPC, cfg.ZPAD], F16)
    cc2_in = nc.dram_tensor("cc2_in", [NPC, cfg.ZPAD], F16)
    z1p = nc.dram_tensor("z1p", [N, cfg.ZPAD], F16, addr_space="Shared")
    z2p = nc.dram_tensor("z2p", [N, cfg.ZPAD], F16, addr_space="Shared")

    ntail = NPC - (NW - 1) * WW  # valid rows in last window

    with tile.TileContext(nc) as tc:
        with tc.tile_pool(name="const", bufs=1) as cpool, \
             tc.tile_pool(name="psum", bufs=8, space="PSUM") as psum_pool, \
             tc.tile_pool(name="sb", bufs=3) as sb, \
             tc.tile_pool(name="gather", bufs=1) as gpool, \
             tc.tile_pool(name="s", bufs=6) as spool, \
             tc.tile_pool(name="idx", bufs=3) as ipool, \
             tc.tile_pool(name="drain", bufs=1) as dpool:

            iota_i = cpool.tile([128, 128], I16)
            nc.gpsimd.iota(iota_i[:], pattern=[[1, 128]], base=0,
                           channel_multiplier=0)
            iota_sb = cpool.tile([128, 128], F16)
            nc.vector.tensor_copy(out=iota_sb[:], in_=iota_i[:])
            W_sb = cpool.tile([F_, C], F32)
            nc.sync.dma_start(out=W_sb[:], in_=W[:, :])
            xT_sb = cpool.tile([F_, NPC], F32)
            nc.sync.dma_start(out=xT_sb[:], in_=xT[:, :])
            d1w_sb = cpool.tile([128, NW], F32)
            nc.sync.dma_start(out=d1w_sb[:], in_=d1w[:, :])
            d2w_sb = cpool.tile([128, NW], F32)
            nc.sync.dma_start(out=d2w_sb[:], in_=d2w[:, :])
            dstw_sb = cpool.tile([128, sched["n_slots"]], F16)
            nc.sync.dma_start(out=dstw_sb[:], in_=dstw[:, :])
            b_sb = cpool.tile([WW, C], F32)
            nc.sync.dma_start(out=b_sb[:], in_=bvec[:, :].to_broadcast([WW, C]))

            # ---- head: Z0 = dinv * (x @ W) -> z0_sb (fp16, padded rows)
            # ---- + cc1_in (padded collective input) ----
            z0_sb = cpool.tile([128, NW, cfg.ZPAD], F16)
            nc.vector.memset(z0_sb[:], 0.0)
            for p in range(NW):
                rows = min(128, NPC - p * 128)
                bank = psum_pool.tile([128, 512], F32, tag="bank")
                nc.tensor.matmul(out=bank[0:rows, 0:C],
                                 lhsT=xT_sb[:, p * 128:p * 128 + rows],
                                 rhs=W_sb[:], start=True, stop=True)
                nc.vector.tensor_scalar_mul(
                    z0_sb[:rows, p, 0:C], bank[0:rows, 0:C],
                    d1w_sb[:rows, p:p + 1])
            nc.sync.dma_start(
                out=cc1_in[0:(NW - 1) * WW, :].rearrange(
                    "(w l) c -> l w c", l=WW),
                in_=z0_sb[:, 0:NW - 1, :])
            nc.sync.dma_start(out=cc1_in[(NW - 1) * WW:NPC, :],
                              in_=z0_sb[0:ntail, NW - 1, :])

            def allgather(cc_in, z_p):
                nc.gpsimd.collective_compute(
                    "AllGather", mybir.AluOpType.bypass,
                    replica_groups=[list(range(cfg.CORES))],
                    ins=[cc_in[:, :].opt()], outs=[z_p[:, :].opt()])

            allgather(cc1_in, z1p)

            pools = dict(psum=psum_pool, gather=gpool, s=spool, idx=ipool)

            # ---- hop1: S1 = edges + self; Z2 = dinv^2 * S1 -> cc2_in ----
            hop1_as = dpool.tile([WW, NW, cfg.ZPAD], F16, tag="asm")
            nc.vector.memset(hop1_as[:], 0.0)
            tsum = dpool.tile([WW, C], F32, tag="tsum")

            def drain1(w_, psum_ap):
                nc.vector.tensor_tensor(out=tsum[:], in0=psum_ap,
                                        in1=z0_sb[:, w_, 0:C],
                                        op=mybir.AluOpType.add)
                nc.vector.tensor_scalar_mul(
                    hop1_as[:, w_, 0:C], tsum[:], d2w_sb[:, w_:w_ + 1])

            emit_hop(nc, tc, cfg, sched, pools, z1p, dstw_sb, iota_sb, gidx,
                     drain1, "hop1")
            nc.sync.dma_start(
                out=cc2_in[0:(NW - 1) * WW, :].rearrange(
                    "(w l) c -> l w c", l=WW),
                in_=hop1_as[:, 0:NW - 1, :])
            nc.sync.dma_start(out=cc2_in[(NW - 1) * WW:NPC, :],
                              in_=hop1_as[0:ntail, NW - 1, :])

            allgather(cc2_in, z2p)

            # ---- hop2: S3 = edges + self; logits = dinv*S3 + b ----
            hop2_as = dpool.tile([WW, NW, C], F32, tag="asm2")
            tsum2 = dpool.tile([WW, C], F32, tag="tsum2")

            def drain2(w_, psum_ap):
                nc.vector.tensor_tensor(out=tsum2[:], in0=psum_ap,
                                        in1=hop1_as[:, w_, 0:C],
                                        op=mybir.AluOpType.add)
                nc.vector.tensor_scalar_mul(
                    hop2_as[:, w_, :], tsum2[:], d1w_sb[:, w_:w_ + 1])
                nc.vector.tensor_add(
                    out=hop2_as[:, w_, :], in0=hop2_as[:, w_, :], in1=b_sb[:])

            emit_hop(nc, tc, cfg, sched, pools, z2p, dstw_sb, iota_sb, gidx,
                     drain2, "hop2")

            # ---- log_softmax over C (free axis) ----
            mx = dpool.tile([WW, NW], F32, tag="mx")
            nc.vector.tensor_reduce(out=mx[:], in_=hop2_as[:],
                                    axis=mybir.AxisListType.X,
                                    op=mybir.AluOpType.max)
            tshift = dpool.tile([WW, NW, C], F32, tag="tshift")
            nc.vector.tensor_tensor(
                out=tshift[:], in0=hop2_as[:],
                in1=mx[:].unsqueeze(2).to_broadcast([WW, NW, C]),
                op=mybir.AluOpType.subtract)
            ex = dpool.tile([WW, NW, C], F32, tag="ex")
            nc.scalar.activation(out=ex[:], in_=tshift[:],
                                 func=mybir.ActivationFunctionType.Exp)
            sm = dpool.tile([WW, NW], F32, tag="sm")
            nc.vector.tensor_reduce(out=sm[:], in_=ex[:],
                                    axis=mybir.AxisListType.X,
                                    op=mybir.AluOpType.add)
            lsm = dpool.tile([WW, NW], F32, tag="lsm")
            nc.scalar.activation(out=lsm[:], in_=sm[:],
                                 func=mybir.ActivationFunctionType.Ln)
            res = dpool.tile([WW, NW, C], F32, tag="res")
            nc.vector.tensor_tensor(
                out=res[:], in0=tshift[:],
                in1=lsm[:].unsqueeze(2).to_broadcast([WW, NW, C]),
                op=mybir.AluOpType.subtract)
            nc.sync.dma_start(
                out=out[0:(NW - 1) * WW, :].rearrange("(w l) c -> l w c", l=WW),
                in_=res[:, 0:NW - 1, :])
            nc.sync.dma_start(out=out[(NW - 1) * WW:NPC, :],
                              in_=res[0:ntail, NW - 1, :])
    nc.compile()
    return nc


def kernel(x, edge_index, W, b, _cfg=None, _trace=False, _sim=False):
    global LAST_RESULTS
    cfg = _cfg or Cfg()
    x = np.asarray(x, dtype=np.float32)
    W_ = np.asarray(W, dtype=np.float32)
    b_ = np.asarray(b, dtype=np.float32).reshape(1, cfg.C)
    sched, per_core = preprocess(np.asarray(edge_index), cfg)
    nc = build_program(cfg, sched)

    in_maps = []
    for c in range(cfg.CORES):
        pc = per_core[c]
        in_maps.append({
            "xT": np.ascontiguousarray(x[c * cfg.NPC:(c + 1) * cfg.NPC, :].T),
            "W": W_, "bvec": b_,
            "d1w": pc["d1w"], "d2w": pc["d2w"],
            "gidx": pc["gidx"], "dstw": pc["dstw"],
        })

    if _sim:
        import concourse.bass_interp as bass_interp
        sim = bass_interp.MultiCoreSim(nc, cfg.CORES)
        for c in range(cfg.CORES):
            for k, v in in_maps[c].items():
                sim.cores[c].tensor(k)[:] = v
        sim.simulate()
        outs = [np.array(sim.cores[c].mem_tensor("out"))
                for c in range(cfg.CORES)]
        return np.concatenate(outs, axis=0)

    if _trace:
        import ntff_shim  # noqa: F401
    res = run_bass_kernel_spmd(nc, in_maps, core_ids=list(range(cfg.CORES)),
                               trace=_trace)
    LAST_RESULTS = res
    return np.concatenate([res.results[c]["out"] for c in range(cfg.CORES)],
                          axis=0)


# revision 34
# speedup vs baseline: 1.2860x; 1.2860x over previous
"""2-hop GCN (gnn_message_passing) on 8 trn2 NeuronCores via Bass.

Algorithm (reference): h = A_hat^2 x;  out = log_softmax(h @ W + b)
  A_hat = D^-1/2 (A + I) D^-1/2  with D = in-degree+1 of (A + I).

Restructured for the device:
  Y  = (x @ W)                     [N, C]   (matmul first: propagation is linear)
  Z0 = dinv * Y                    (node-wise scale)
  S1 = sum_{edges} Z0[src] + Z0[t] (self-loop added at drain, not gathered)
  Z2 = dinv^2 * S1
  S3 = same with Z2
  out = log_softmax(dinv * S3 + b)

Sharding: nodes (and edges by target) across 8 cores; full Z exchanged
between hops via AllGather collectives. Propagation data in fp16 (PSUM
accumulation exact fp32); head/tail in fp32.

Per-hop device pattern per core:
  - dma_gather (SWDGE) pulls Z[src] rows (fp16, 80B payload, 256B-strided
    source) for edge chunks of 128, batched 24 chunks (3072 idx) per
    instruction with single_packet=False (with single_packet=True the
    per-engine descriptor stream must stay under ~64 descs = 1024 idxs,
    or the device dies).
  - Edges are pre-sorted by (src-half, target window of 128); per target
    window the one-hot S tiles are built in one batched DVE is_equal per
    (half, window) run, then PE matmuls S^T @ msgs accumulate the window's
    PSUM bank.
  - Windows drain with (bank + Z_local) * dinv-scale fused on DVE.
"""
import math

import numpy as np

import concourse.bacc as bacc
import concourse.bass as bass
import concourse.mybir as mybir
import concourse.tile as tile
from concourse.bass_utils import run_bass_kernel_spmd
from concourse.masks import make_identity

F32 = mybir.dt.float32
F16 = mybir.dt.float16
I16 = mybir.dt.int16

LAST_RESULTS = None  # test harness peeks at this after kernel(..., _trace=True)

N_SWDGE_QUEUES = 2


class Cfg:
    def __init__(self, N=50000, F=100, C=40, CORES=8, W_WIN=128, BATCH_SLOTS=24,
                 ZPAD=128):
        self.N, self.F, self.C, self.CORES = N, F, C, CORES
        assert N % CORES == 0
        self.NPC = N // CORES
        assert N % 2 == 0
        self.HALF = N // 2
        assert self.HALF <= 32768, "gather idx must fit int16"
        self.W_WIN = W_WIN                      # targets per PSUM window
        self.N_WIN = math.ceil(self.NPC / W_WIN)
        self.BATCH_SLOTS = BATCH_SLOTS          # chunks per gather instruction
        self.BIDX = BATCH_SLOTS * 128           # idxs per gather instruction
        self.ZPAD = ZPAD                        # fp16 row stride of Z (256B)
        assert (ZPAD * 2) % 256 == 0
        self.NT_HEAD = math.ceil(self.NPC / 128)


def _wrap_idx(flat):
    """idx position i -> [i%16, i//16], replicated to 128 partitions."""
    n = flat.shape[0]
    assert n % 16 == 0
    w = flat.reshape(n // 16, 16).T  # [16, n/16]
    return np.tile(w, (8, 1))        # [128, n/16]


def preprocess(edge_index, cfg):
    """Build the shared compile-time schedule and per-core device arrays."""
    N, NPC, W_WIN, N_WIN = cfg.N, cfg.NPC, cfg.W_WIN, cfg.N_WIN
    CH = 128
    row = np.asarray(edge_index[0], dtype=np.int64)
    col = np.asarray(edge_index[1], dtype=np.int64)

    deg = np.bincount(col, minlength=N).astype(np.float64) + 1.0
    dinv = 1.0 / np.sqrt(deg)

    cores = []
    counts = np.zeros((cfg.CORES, 2, N_WIN), np.int64)
    for c in range(cfg.CORES):
        lo = c * NPC
        m = (col >= lo) & (col < lo + NPC)
        s = row[m]
        d = col[m] - lo
        h = (s >= cfg.HALF).astype(np.int64)
        w = d // W_WIN
        order = np.lexsort((d, w, h))
        s, d, h, w = s[order], d[order], h[order], w[order]
        cores.append((s, d, h, w))
        for half in (0, 1):
            counts[c, half] = np.bincount(w[h == half], minlength=N_WIN)

    cpg = -(-counts // CH)      # ceil
    cpg = cpg.max(axis=0)       # [2, N_WIN] chunks per (half, window)

    slots = []                  # list of (half, win)
    for half in (0, 1):
        for w_ in range(N_WIN):
            slots += [(half, w_)] * int(cpg[half, w_])
        pad = (-len(slots)) % max(cfg.BATCH_SLOTS, 16)
        slots += [(half, N_WIN - 1)] * pad
    n_slots = len(slots)
    n_batches = n_slots // cfg.BATCH_SLOTS
    batch_half = [slots[b * cfg.BATCH_SLOTS][0] for b in range(n_batches)]
    for b in range(n_batches):
        assert all(s[0] == batch_half[b]
                   for s in slots[b * cfg.BATCH_SLOTS:(b + 1) * cfg.BATCH_SLOTS])

    # group -> its slot indices (gather order: half-major for batch purity).
    # Within a group, slot ids are a consecutive run.
    group_slots = {}
    for j, (h_, w_) in enumerate(slots):
        group_slots.setdefault((h_, w_), []).append(j)
    for sl in group_slots.values():
        assert sl == list(range(sl[0], sl[0] + len(sl)))

    # matmul order: window-major (one PSUM bank per window, <=8 in flight);
    # each window is up to two consecutive slot runs (half 0, half 1)
    win_runs = [[(sl[0], len(sl)) for sl in
                 (group_slots.get((0, w_)), group_slots.get((1, w_))) if sl]
                for w_ in range(N_WIN)]
    sched = dict(slots=slots, n_slots=n_slots, n_batches=n_batches,
                 batch_half=batch_half, win_runs=win_runs)

    per_core = []
    for c in range(cfg.CORES):
        s, d, h, w = cores[c]
        gidx = np.zeros(n_slots * CH, np.int16)
        dstw = np.full((CH, n_slots), 255.0, np.float16)
        for half in (0, 1):
            hm = h == half
            sh, dh, wh = s[hm], d[hm], w[hm]
            for w_ in range(N_WIN):
                wm = wh == w_
                se = sh[wm] - half * cfg.HALF
                de = dh[wm] - w_ * W_WIN
                sl = group_slots[(half, w_)]
                for k in range(se.shape[0]):
                    j = sl[k // CH]
                    lane = k % CH
                    gidx[j * CH + lane] = se[k]
                    dstw[lane, j] = de[k]
        gidx_w = np.concatenate(
            [_wrap_idx(gidx[b * cfg.BIDX:(b + 1) * cfg.BIDX])
             for b in range(n_batches)], axis=0)  # [n_batches*128, BIDX/16]

        lo = c * NPC
        dl = dinv[lo:lo + NPC]
        d1w = np.zeros((128, N_WIN), np.float32)
        for p in range(N_WIN):
            r = min(128, NPC - p * 128)
            d1w[:r, p] = dl[p * 128:p * 128 + r]
        per_core.append(dict(gidx=gidx_w, dstw=dstw,
                             d1w=d1w, d2w=d1w * d1w))
    return sched, per_core


def dma_gather_raw(nc, out_ap, in_ap, idxs_ap, num_idxs, elem_size, elem_step,
                   queue_num=0):
    """dma_gather without the blanket elem_size%256B assert (payload may be
    byte-granular; only the source row stride is 256B-encoded)."""
    eng = nc.gpsimd
    dts = mybir.dt.size(in_ap.dtype)
    stride_bytes = elem_step * dts
    assert stride_bytes % 256 == 0 and stride_bytes // 256 < 256
    assert in_ap.ap[0][0] == elem_step and in_ap.ap[-1][1] == elem_size
    assert idxs_ap.tensor.dtype == I16
    _in_ap = eng.lower_ap_dma(in_ap, for_custom_bir_dma=True)
    _idxs_ap = eng.lower_ap(idxs_ap)
    _out_ap = eng.lower_ap(out_ap)
    return eng.add_instruction(mybir.InstDMAGatherAnt(
        name=nc.get_next_instruction_name(),
        ins=[*_in_ap, _idxs_ap, eng.lower_val_access(eng.to_reg(num_idxs))],
        outs=[_out_ap],
        transpose=False, num_idxs=num_idxs, elem_size=elem_size,
        stride_bytes_256=stride_bytes // 256, gen_mode=0, single_packet=False,
        queue_num=queue_num, sbuf_tokens_per_rank=0, sbuf_free_dim_per_rank=0,
        sbuf_free_dim_pad_per_rank=0, sbuf_byte_offset=0))


def emit_hop(nc, tc, cfg, sched, pools, z_pad, dstw_sb, iota_sb, gidx_dram,
             out_assemble, hop_tag):
    """One propagation hop: gather + one-hot matmul segment-sum + drain.

    z_pad: DRAM [N, ZPAD] fp16 gather source.
    out_assemble: callback(w, src_psum_ap) emitting the drain for window w.
    """
    C, WW = cfg.C, cfg.W_WIN
    psum_pool, gpool, spool, ipool = (pools[k] for k in
                                      ("psum", "gather", "s", "idx"))
    # gather ALL batches up front (tiles stay resident; matmuls consume in
    # window order while later batches still stream in)
    msgs = []
    for b in range(sched["n_batches"]):
        h = sched["batch_half"][b]
        idx_sb = ipool.tile([128, cfg.BIDX // 16], I16, tag="idx")
        nc.sync.dma_start(
            out=idx_sb[:],
            in_=gidx_dram[b * 128:(b + 1) * 128, :])
        msg = gpool.tile([128, cfg.BATCH_SLOTS, C], F16, tag=f"msg{b}")
        src_view = z_pad[h * cfg.HALF:(h + 1) * cfg.HALF, 0:C]
        dma_gather_raw(nc, msg[:], src_view, idx_sb[:], cfg.BIDX,
                       elem_size=C, elem_step=cfg.ZPAD,
                       queue_num=b % N_SWDGE_QUEUES)
        msgs.append(msg)

    # one PSUM bank per window; pool rotation keeps <=8 windows in flight
    for w_ in range(cfg.N_WIN):
        bank = psum_pool.tile([128, 512], F32, tag="bank",
                              name=f"{hop_tag}_bank_w{w_}")
        runs = sched["win_runs"][w_]
        nrun = sum(ln for _, ln in runs)
        k = 0
        for j0, ln in runs:
            # batched one-hot build: S[:, i, :] one-hot of dstw col j0+i
            S = spool.tile([128, ln, WW], F16, tag="S")
            nc.vector.tensor_tensor(
                out=S[:],
                in0=iota_sb[:, :WW].unsqueeze(1).to_broadcast([128, ln, WW]),
                in1=dstw_sb[:, j0:j0 + ln].unsqueeze(2).to_broadcast(
                    [128, ln, WW]),
                op=mybir.AluOpType.is_equal)
            for i in range(ln):
                j = j0 + i
                b, q = j // cfg.BATCH_SLOTS, j % cfg.BATCH_SLOTS
                nc.tensor.matmul(
                    out=bank[0:WW, 0:C],
                    lhsT=S[:, i, :], rhs=msgs[b][:, q, :],
                    start=(k == 0),
                    stop=(k == nrun - 1))
                k += 1
        out_assemble(w_, bank[0:WW, 0:C])


def build_program(cfg, sched):
    nc = bacc.Bacc("TRN2", target_bir_lowering=False, debug=False,
                   num_devices=cfg.CORES, num_swdge_queues=N_SWDGE_QUEUES)
    N, F_, C, NPC, WW, NW = cfg.N, cfg.F, cfg.C, cfg.NPC, cfg.W_WIN, cfg.N_WIN

    xT = nc.dram_tensor("xT", [F_, NPC], F32, kind="ExternalInput")
    W = nc.dram_tensor("W", [F_, C], F32, kind="ExternalInput")
    bvec = nc.dram_tensor("bvec", [1, C], F32, kind="ExternalInput")
    d1w = nc.dram_tensor("d1w", [128, NW], F32, kind="ExternalInput")
    d2w = nc.dram_tensor("d2w", [128, NW], F32, kind="ExternalInput")
    gidx = nc.dram_tensor("gidx", [sched["n_batches"] * 128, cfg.BIDX // 16],
                          I16, kind="ExternalInput")
    dstw = nc.dram_tensor("dstw", [128, sched["n_slots"]], F16,
                          kind="ExternalInput")
    out = nc.dram_tensor("out", [NPC, C], F32, kind="ExternalOutput")

    # Collectives run on the 256B-padded row layout directly: the AllGather
    # output IS the gather source, killing the 50k-descriptor pad copy that
    # otherwise sits serially on the critical path (~190us per hop).
    cc1_in = nc.dram_tensor("cc1_in", [NPC, cfg.ZPAD], F16)
    cc2_in = nc.dram_tensor("cc2_in", [NPC, cfg.ZPAD], F16)
    z1p = nc.dram_tensor("z1p", [N, cfg.ZPAD], F16, addr_space="Shared")
    z2p = nc.dram_tensor("z2p", [N, cfg.ZPAD], F16, addr_space="Shared")

    ntail = NPC - (NW - 1) * WW  # valid rows in last window

    with tile.TileContext(nc) as tc:
        with tc.tile_pool(name="const", bufs=1) as cpool, \
             tc.tile_pool(name="psum", bufs=8, space="PSUM") as psum_pool, \
             tc.tile_pool(name="sb", bufs=3) as sb, \
             tc.tile_pool(name="gather", bufs=1) as gpool, \
             tc.tile_pool(name="s", bufs=6) as spool, \
             tc.tile_pool(name="idx", bufs=3) as ipool, \
             tc.tile_pool(name="drain", bufs=1) as dpool:

            iota_i = cpool.tile([128, 128], I16)
            nc.gpsimd.iota(iota_i[:], pattern=[[1, 128]], base=0,
                           channel_multiplier=0)
            iota_sb = cpool.tile([128, 128], F16)
            nc.vector.tensor_copy(out=iota_sb[:], in_=iota_i[:])
            W_sb = cpool.tile([F_, C], F32)
            nc.sync.dma_start(out=W_sb[:], in_=W[:, :])
            xT_sb = cpool.tile([F_, NPC], F32)
            nc.sync.dma_start(out=xT_sb[:], in_=xT[:, :])
            d1w_sb = cpool.tile([128, NW], F32)
            nc.sync.dma_start(out=d1w_sb[:], in_=d1w[:, :])
            d2w_sb = cpool.tile([128, NW], F32)
            nc.sync.dma_start(out=d2w_sb[:], in_=d2w[:, :])
            dstw_sb = cpool.tile([128, sched["n_slots"]], F16)
            nc.sync.dma_start(out=dstw_sb[:], in_=dstw[:, :])
            b_sb = cpool.tile([WW, C], F32)
            nc.sync.dma_start(out=b_sb[:], in_=bvec[:, :].to_broadcast([WW, C]))

            # ---- head: Z0 = dinv * (x @ W) -> z0_sb (fp16, padded rows)
            # ---- + cc1_in (padded collective input) ----
            z0_sb = cpool.tile([128, NW, cfg.ZPAD], F16)
            nc.vector.memset(z0_sb[:], 0.0)
            for p in range(NW):
                rows = min(128, NPC - p * 128)
                bank = psum_pool.tile([128, 512], F32, tag="bank")
                nc.tensor.matmul(out=bank[0:rows, 0:C],
                                 lhsT=xT_sb[:, p * 128:p * 128 + rows],
                                 rhs=W_sb[:], start=True, stop=True)
                nc.vector.tensor_scalar_mul(
                    z0_sb[:rows, p, 0:C], bank[0:rows, 0:C],
                    d1w_sb[:rows, p:p + 1])
            nc.sync.dma_start(
                out=cc1_in[0:(NW - 1) * WW, :].rearrange(
                    "(w l) c -> l w c", l=WW),
                in_=z0_sb[:, 0:NW - 1, :])
            nc.sync.dma_start(out=cc1_in[(NW - 1) * WW:NPC, :],
                              in_=z0_sb[0:ntail, NW - 1, :])

            def allgather(cc_in, z_p):
                nc.gpsimd.collective_compute(
                    "AllGather", mybir.AluOpType.bypass,
                    replica_groups=[list(range(cfg.CORES))],
                    ins=[cc_in[:, :].opt()], outs=[z_p[:, :].opt()])

            allgather(cc1_in, z1p)

            pools = dict(psum=psum_pool, gather=gpool, s=spool, idx=ipool)

            # ---- hop1: S1 = edges + self; Z2 = dinv^2 * S1 -> cc2_in ----
            hop1_as = dpool.tile([WW, NW, cfg.ZPAD], F16, tag="asm")
            nc.vector.memset(hop1_as[:], 0.0)
            tsum = dpool.tile([WW, C], F32, tag="tsum")

            def drain1(w_, psum_ap):
                nc.vector.tensor_tensor(out=tsum[:], in0=psum_ap,
                                        in1=z0_sb[:, w_, 0:C],
                                        op=mybir.AluOpType.add)
                nc.vector.tensor_scalar_mul(
                    hop1_as[:, w_, 0:C], tsum[:], d2w_sb[:, w_:w_ + 1])

            emit_hop(nc, tc, cfg, sched, pools, z1p, dstw_sb, iota_sb, gidx,
                     drain1, "hop1")
            nc.sync.dma_start(
                out=cc2_in[0:(NW - 1) * WW, :].rearrange(
                    "(w l) c -> l w c", l=WW),
                in_=hop1_as[:, 0:NW - 1, :])
            nc.sync.dma_start(out=cc2_in[(NW - 1) * WW:NPC, :],
                              in_=hop1_as[0:ntail, NW - 1, :])

            allgather(cc2_in, z2p)

            # ---- hop2: S3 = edges + self; logits = dinv*S3 + b ----
            hop2_as = dpool.tile([WW, NW, C], F32, tag="asm2")
            tsum2 = dpool.tile([WW, C], F32, tag="tsum2")

            def drain2(w_, psum_ap):
                nc.vector.tensor_tensor(out=tsum2[:], in0=psum_ap,
                                        in1=hop1_as[:, w_, 0:C],
                                        op=mybir.AluOpType.add)
                nc.vector.tensor_scalar_mul(
                    hop2_as[:, w_, :], tsum2[:], d1w_sb[:, w_:w_ + 1])
                nc.vector.tensor_add(
                    out=hop2_as[:, w_, :], in0=hop2_as[:, w_, :], in1=b_sb[:])

            emit_hop(nc, tc, cfg, sched, pools, z2p, dstw_sb, iota_sb, gidx,
                     drain2, "hop2")

            # ---- log_softmax over C (free axis) ----
            mx = dpool.tile([WW, NW], F32, tag="mx")
            nc.vector.tensor_reduce(out=mx[:], in_=hop2_as[:],
                                    axis=mybir.AxisListType.X,
                                    op=mybir.AluOpType.max)
            tshift = dpool.tile([WW, NW, C], F32, tag="tshift")
            nc.vector.tensor_tensor(
                out=tshift[:], in0=hop2_as[:],
                in1=mx[:].unsqueeze(2).to_broadcast([WW, NW, C]),
                op=mybir.AluOpType.subtract)
            ex = dpool.tile([WW, NW, C], F32, tag="ex")
            nc.scalar.activation(out=ex[:], in_=tshift[:],
                                 func=mybir.ActivationFunctionType.Exp)
            sm = dpool.tile([WW, NW], F32, tag="sm")
            nc.vector.tensor_reduce(out=sm[:], in_=ex[:],
                                    axis=mybir.AxisListType.X,
                                    op=mybir.AluOpType.add)
            lsm = dpool.tile([WW, NW], F32, tag="lsm")
            nc.scalar.activation(out=lsm[:], in_=sm[:],
                                 func=mybir.ActivationFunctionType.Ln)
            res = dpool.tile([WW, NW, C], F32, tag="res")
            nc.vector.tensor_tensor(
                out=res[:], in0=tshift[:],
                in1=lsm[:].unsqueeze(2).to_broadcast([WW, NW, C]),
                op=mybir.AluOpType.subtract)
            nc.sync.dma_start(
                out=out[0:(NW - 1) * WW, :].rearrange("(w l) c -> l w c", l=WW),
                in_=res[:, 0:NW - 1, :])
            nc.sync.dma_start(out=out[(NW - 1) * WW:NPC, :],
                              in_=res[0:ntail, NW - 1, :])
    nc.compile()
    return nc


def kernel(x, edge_index, W, b, _cfg=None, _trace=False, _sim=False):
    global LAST_RESULTS
    cfg = _cfg or Cfg()
    x = np.asarray(x, dtype=np.float32)
    W_ = np.asarray(W, dtype=np.float32)
    b_ = np.asarray(b, dtype=np.float32).reshape(1, cfg.C)
    sched, per_core = preprocess(np.asarray(edge_index), cfg)
    nc = build_program(cfg, sched)

    in_maps = []
    for c in range(cfg.CORES):
        pc = per_core[c]
        in_maps.append({
            "xT": np.ascontiguousarray(x[c * cfg.NPC:(c + 1) * cfg.NPC, :].T),
            "W": W_, "bvec": b_,
            "d1w": pc["d1w"], "d2w": pc["d2w"],
            "gidx": pc["gidx"], "dstw": pc["dstw"],
        })

    if _sim:
        import concourse.bass_interp as bass_interp
        sim = bass_interp.MultiCoreSim(nc, cfg.CORES)
        for c in range(cfg.CORES):
            for k, v in in_maps[c].items():
                sim.cores[c].tensor(k)[:] = v
        sim.simulate()
        outs = [np.array(sim.cores[c].mem_tensor("out"))
                for c in range(cfg.CORES)]
        return np.concatenate(outs, axis=0)

    if _trace:
        import ntff_shim  # noqa: F401
    res = run_bass_kernel_spmd(nc, in_maps, core_ids=list(range(cfg.CORES)),
                               trace=_trace)
    LAST_RESULTS = res
    return np.concatenate([res.results[c]["out"] for c in range(cfg.CORES)],
                          axis=0)


# revision 35
# speedup vs baseline: 1.4534x; 1.1302x over previous
"""2-hop GCN (gnn_message_passing) on 8 trn2 NeuronCores via Bass.

Algorithm (reference): h = A_hat^2 x;  out = log_softmax(h @ W + b)
  A_hat = D^-1/2 (A + I) D^-1/2  with D = in-degree+1 of (A + I).

Restructured for the device:
  Y  = (x @ W)                     [N, C]   (matmul first: propagation is linear)
  Z0 = dinv * Y                    (node-wise scale)
  S1 = sum_{edges} Z0[src] + Z0[t] (self-loop added at drain, not gathered)
  Z2 = dinv^2 * S1
  S3 = same with Z2
  out = log_softmax(dinv * S3 + b)

Sharding: nodes (and edges by target) across 8 cores; full Z exchanged
between hops via AllGather collectives. Propagation data in fp16 (PSUM
accumulation exact fp32); head/tail in fp32.

Per-hop device pattern per core:
  - dma_gather (SWDGE) pulls Z[src] rows (fp16, 80B payload, 256B-strided
    source) for edge chunks of 128, batched 24 chunks (3072 idx) per
    instruction with single_packet=False (with single_packet=True the
    per-engine descriptor stream must stay under ~64 descs = 1024 idxs,
    or the device dies).
  - Edges are pre-sorted by (src-half, target window of 128); per target
    window the one-hot S tiles are built in one batched DVE is_equal per
    (half, window) run, then PE matmuls S^T @ msgs accumulate the window's
    PSUM bank.
  - Windows drain with (bank + Z_local) * dinv-scale fused on DVE.
"""
import math

import numpy as np

import concourse.bacc as bacc
import concourse.bass as bass
import concourse.mybir as mybir
import concourse.tile as tile
from concourse.bass_utils import run_bass_kernel_spmd
from concourse.masks import make_identity

F32 = mybir.dt.float32
F16 = mybir.dt.float16
I16 = mybir.dt.int16

LAST_RESULTS = None  # test harness peeks at this after kernel(..., _trace=True)

N_SWDGE_QUEUES = 4


class Cfg:
    def __init__(self, N=50000, F=100, C=40, CORES=8, W_WIN=128, BATCH_SLOTS=24,
                 ZPAD=128):
        self.N, self.F, self.C, self.CORES = N, F, C, CORES
        assert N % CORES == 0
        self.NPC = N // CORES
        assert N % 2 == 0
        self.HALF = N // 2
        assert self.HALF <= 32768, "gather idx must fit int16"
        self.W_WIN = W_WIN                      # targets per PSUM window
        self.N_WIN = math.ceil(self.NPC / W_WIN)
        self.BATCH_SLOTS = BATCH_SLOTS          # chunks per gather instruction
        self.BIDX = BATCH_SLOTS * 128           # idxs per gather instruction
        self.ZPAD = ZPAD                        # fp16 row stride of Z (256B)
        assert (ZPAD * 2) % 256 == 0
        self.NT_HEAD = math.ceil(self.NPC / 128)


def _wrap_idx(flat):
    """idx position i -> [i%16, i//16], replicated to 128 partitions."""
    n = flat.shape[0]
    assert n % 16 == 0
    w = flat.reshape(n // 16, 16).T  # [16, n/16]
    return np.tile(w, (8, 1))        # [128, n/16]


def preprocess(edge_index, cfg):
    """Build the shared compile-time schedule and per-core device arrays."""
    N, NPC, W_WIN, N_WIN = cfg.N, cfg.NPC, cfg.W_WIN, cfg.N_WIN
    CH = 128
    row = np.asarray(edge_index[0], dtype=np.int64)
    col = np.asarray(edge_index[1], dtype=np.int64)

    deg = np.bincount(col, minlength=N).astype(np.float64) + 1.0
    dinv = 1.0 / np.sqrt(deg)

    cores = []
    counts = np.zeros((cfg.CORES, 2, N_WIN), np.int64)
    for c in range(cfg.CORES):
        lo = c * NPC
        m = (col >= lo) & (col < lo + NPC)
        s = row[m]
        d = col[m] - lo
        h = (s >= cfg.HALF).astype(np.int64)
        w = d // W_WIN
        order = np.lexsort((d, w, h))
        s, d, h, w = s[order], d[order], h[order], w[order]
        cores.append((s, d, h, w))
        for half in (0, 1):
            counts[c, half] = np.bincount(w[h == half], minlength=N_WIN)

    cpg = -(-counts // CH)      # ceil
    cpg = cpg.max(axis=0)       # [2, N_WIN] chunks per (half, window)

    slots = []                  # list of (half, win)
    for half in (0, 1):
        for w_ in range(N_WIN):
            slots += [(half, w_)] * int(cpg[half, w_])
        pad = (-len(slots)) % max(cfg.BATCH_SLOTS, 16)
        slots += [(half, N_WIN - 1)] * pad
    n_slots = len(slots)
    n_batches = n_slots // cfg.BATCH_SLOTS
    batch_half = [slots[b * cfg.BATCH_SLOTS][0] for b in range(n_batches)]
    for b in range(n_batches):
        assert all(s[0] == batch_half[b]
                   for s in slots[b * cfg.BATCH_SLOTS:(b + 1) * cfg.BATCH_SLOTS])

    # group -> its slot indices (gather order: half-major for batch purity).
    # Within a group, slot ids are a consecutive run.
    group_slots = {}
    for j, (h_, w_) in enumerate(slots):
        group_slots.setdefault((h_, w_), []).append(j)
    for sl in group_slots.values():
        assert sl == list(range(sl[0], sl[0] + len(sl)))

    # matmul order: window-major (one PSUM bank per window, <=8 in flight);
    # each window is up to two consecutive slot runs (half 0, half 1)
    win_runs = [[(sl[0], len(sl)) for sl in
                 (group_slots.get((0, w_)), group_slots.get((1, w_))) if sl]
                for w_ in range(N_WIN)]
    sched = dict(slots=slots, n_slots=n_slots, n_batches=n_batches,
                 batch_half=batch_half, win_runs=win_runs)

    per_core = []
    for c in range(cfg.CORES):
        s, d, h, w = cores[c]
        gidx = np.zeros(n_slots * CH, np.int16)
        dstw = np.full((CH, n_slots), 255.0, np.float16)
        for half in (0, 1):
            hm = h == half
            sh, dh, wh = s[hm], d[hm], w[hm]
            for w_ in range(N_WIN):
                wm = wh == w_
                se = sh[wm] - half * cfg.HALF
                de = dh[wm] - w_ * W_WIN
                sl = group_slots[(half, w_)]
                for k in range(se.shape[0]):
                    j = sl[k // CH]
                    lane = k % CH
                    gidx[j * CH + lane] = se[k]
                    dstw[lane, j] = de[k]
        gidx_w = np.concatenate(
            [_wrap_idx(gidx[b * cfg.BIDX:(b + 1) * cfg.BIDX])
             for b in range(n_batches)], axis=0)  # [n_batches*128, BIDX/16]

        lo = c * NPC
        dl = dinv[lo:lo + NPC]
        d1w = np.zeros((128, N_WIN), np.float32)
        for p in range(N_WIN):
            r = min(128, NPC - p * 128)
            d1w[:r, p] = dl[p * 128:p * 128 + r]
        per_core.append(dict(gidx=gidx_w, dstw=dstw,
                             d1w=d1w, d2w=d1w * d1w))
    return sched, per_core


def dma_gather_raw(nc, out_ap, in_ap, idxs_ap, num_idxs, elem_size, elem_step,
                   queue_num=0):
    """dma_gather without the blanket elem_size%256B assert (payload may be
    byte-granular; only the source row stride is 256B-encoded)."""
    eng = nc.gpsimd
    dts = mybir.dt.size(in_ap.dtype)
    stride_bytes = elem_step * dts
    assert stride_bytes % 256 == 0 and stride_bytes // 256 < 256
    assert in_ap.ap[0][0] == elem_step and in_ap.ap[-1][1] == elem_size
    assert idxs_ap.tensor.dtype == I16
    _in_ap = eng.lower_ap_dma(in_ap, for_custom_bir_dma=True)
    _idxs_ap = eng.lower_ap(idxs_ap)
    _out_ap = eng.lower_ap(out_ap)
    return eng.add_instruction(mybir.InstDMAGatherAnt(
        name=nc.get_next_instruction_name(),
        ins=[*_in_ap, _idxs_ap, eng.lower_val_access(eng.to_reg(num_idxs))],
        outs=[_out_ap],
        transpose=False, num_idxs=num_idxs, elem_size=elem_size,
        stride_bytes_256=stride_bytes // 256, gen_mode=0, single_packet=False,
        queue_num=queue_num, sbuf_tokens_per_rank=0, sbuf_free_dim_per_rank=0,
        sbuf_free_dim_pad_per_rank=0, sbuf_byte_offset=0))


def emit_hop(nc, tc, cfg, sched, pools, z_pad, dstw_sb, iota_sb, gidx_dram,
             out_assemble, hop_tag):
    """One propagation hop: gather + one-hot matmul segment-sum + drain.

    z_pad: DRAM [N, ZPAD] fp16 gather source.
    out_assemble: callback(w, src_psum_ap) emitting the drain for window w.
    """
    C, WW = cfg.C, cfg.W_WIN
    psum_pool, gpool, spool, ipool = (pools[k] for k in
                                      ("psum", "gather", "s", "idx"))
    # gather ALL batches up front (tiles stay resident; matmuls consume in
    # window order while later batches still stream in)
    msgs = []
    for b in range(sched["n_batches"]):
        h = sched["batch_half"][b]
        idx_sb = ipool.tile([128, cfg.BIDX // 16], I16, tag="idx")
        nc.sync.dma_start(
            out=idx_sb[:],
            in_=gidx_dram[b * 128:(b + 1) * 128, :])
        msg = gpool.tile([128, cfg.BATCH_SLOTS, C], F16, tag=f"msg{b}")
        src_view = z_pad[h * cfg.HALF:(h + 1) * cfg.HALF, 0:C]
        dma_gather_raw(nc, msg[:], src_view, idx_sb[:], cfg.BIDX,
                       elem_size=C, elem_step=cfg.ZPAD,
                       queue_num=b % N_SWDGE_QUEUES)
        msgs.append(msg)

    # one PSUM bank per window; pool rotation keeps <=8 windows in flight
    for w_ in range(cfg.N_WIN):
        bank = psum_pool.tile([128, 512], F32, tag="bank",
                              name=f"{hop_tag}_bank_w{w_}")
        runs = sched["win_runs"][w_]
        nrun = sum(ln for _, ln in runs)
        k = 0
        for j0, ln in runs:
            # batched one-hot build: S[:, i, :] one-hot of dstw col j0+i
            S = spool.tile([128, ln, WW], F16, tag="S")
            nc.vector.tensor_tensor(
                out=S[:],
                in0=iota_sb[:, :WW].unsqueeze(1).to_broadcast([128, ln, WW]),
                in1=dstw_sb[:, j0:j0 + ln].unsqueeze(2).to_broadcast(
                    [128, ln, WW]),
                op=mybir.AluOpType.is_equal)
            for i in range(ln):
                j = j0 + i
                b, q = j // cfg.BATCH_SLOTS, j % cfg.BATCH_SLOTS
                nc.tensor.matmul(
                    out=bank[0:WW, 0:C],
                    lhsT=S[:, i, :], rhs=msgs[b][:, q, :],
                    start=(k == 0),
                    stop=(k == nrun - 1))
                k += 1
        out_assemble(w_, bank[0:WW, 0:C])


def build_program(cfg, sched):
    nc = bacc.Bacc("TRN2", target_bir_lowering=False, debug=False,
                   num_devices=cfg.CORES, num_swdge_queues=N_SWDGE_QUEUES)
    N, F_, C, NPC, WW, NW = cfg.N, cfg.F, cfg.C, cfg.NPC, cfg.W_WIN, cfg.N_WIN

    xT = nc.dram_tensor("xT", [F_, NPC], F32, kind="ExternalInput")
    W = nc.dram_tensor("W", [F_, C], F32, kind="ExternalInput")
    bvec = nc.dram_tensor("bvec", [1, C], F32, kind="ExternalInput")
    d1w = nc.dram_tensor("d1w", [128, NW], F32, kind="ExternalInput")
    d2w = nc.dram_tensor("d2w", [128, NW], F32, kind="ExternalInput")
    gidx = nc.dram_tensor("gidx", [sched["n_batches"] * 128, cfg.BIDX // 16],
                          I16, kind="ExternalInput")
    dstw = nc.dram_tensor("dstw", [128, sched["n_slots"]], F16,
                          kind="ExternalInput")
    out = nc.dram_tensor("out", [NPC, C], F32, kind="ExternalOutput")

    # Collectives run on the 256B-padded row layout directly: the AllGather
    # output IS the gather source, killing the 50k-descriptor pad copy that
    # otherwise sits serially on the critical path (~190us per hop).
    cc1_in = nc.dram_tensor("cc1_in", [NPC, cfg.ZPAD], F16)
    cc2_in = nc.dram_tensor("cc2_in", [NPC, cfg.ZPAD], F16)
    z1p = nc.dram_tensor("z1p", [N, cfg.ZPAD], F16, addr_space="Shared")
    z2p = nc.dram_tensor("z2p", [N, cfg.ZPAD], F16, addr_space="Shared")

    ntail = NPC - (NW - 1) * WW  # valid rows in last window

    with tile.TileContext(nc) as tc:
        with tc.tile_pool(name="const", bufs=1) as cpool, \
             tc.tile_pool(name="psum", bufs=8, space="PSUM") as psum_pool, \
             tc.tile_pool(name="sb", bufs=3) as sb, \
             tc.tile_pool(name="gather", bufs=1) as gpool, \
             tc.tile_pool(name="s", bufs=6) as spool, \
             tc.tile_pool(name="idx", bufs=3) as ipool, \
             tc.tile_pool(name="drain", bufs=1) as dpool:

            iota_i = cpool.tile([128, 128], I16)
            nc.gpsimd.iota(iota_i[:], pattern=[[1, 128]], base=0,
                           channel_multiplier=0)
            iota_sb = cpool.tile([128, 128], F16)
            nc.vector.tensor_copy(out=iota_sb[:], in_=iota_i[:])
            W_sb = cpool.tile([F_, C], F32)
            nc.sync.dma_start(out=W_sb[:], in_=W[:, :])
            xT_sb = cpool.tile([F_, NPC], F32)
            nc.sync.dma_start(out=xT_sb[:], in_=xT[:, :])
            d1w_sb = cpool.tile([128, NW], F32)
            nc.sync.dma_start(out=d1w_sb[:], in_=d1w[:, :])
            d2w_sb = cpool.tile([128, NW], F32)
            nc.sync.dma_start(out=d2w_sb[:], in_=d2w[:, :])
            dstw_sb = cpool.tile([128, sched["n_slots"]], F16)
            nc.sync.dma_start(out=dstw_sb[:], in_=dstw[:, :])
            b_sb = cpool.tile([WW, C], F32)
            nc.sync.dma_start(out=b_sb[:], in_=bvec[:, :].to_broadcast([WW, C]))

            # ---- head: Z0 = dinv * (x @ W) -> z0_sb (fp16, padded rows)
            # ---- + cc1_in (padded collective input) ----
            z0_sb = cpool.tile([128, NW, cfg.ZPAD], F16)
            nc.vector.memset(z0_sb[:], 0.0)
            for p in range(NW):
                rows = min(128, NPC - p * 128)
                bank = psum_pool.tile([128, 512], F32, tag="bank")
                nc.tensor.matmul(out=bank[0:rows, 0:C],
                                 lhsT=xT_sb[:, p * 128:p * 128 + rows],
                                 rhs=W_sb[:], start=True, stop=True)
                nc.vector.tensor_scalar_mul(
                    z0_sb[:rows, p, 0:C], bank[0:rows, 0:C],
                    d1w_sb[:rows, p:p + 1])
            nc.sync.dma_start(
                out=cc1_in[0:(NW - 1) * WW, :].rearrange(
                    "(w l) c -> l w c", l=WW),
                in_=z0_sb[:, 0:NW - 1, :])
            nc.sync.dma_start(out=cc1_in[(NW - 1) * WW:NPC, :],
                              in_=z0_sb[0:ntail, NW - 1, :])

            def allgather(cc_in, z_p):
                nc.gpsimd.collective_compute(
                    "AllGather", mybir.AluOpType.bypass,
                    replica_groups=[list(range(cfg.CORES))],
                    ins=[cc_in[:, :].opt()], outs=[z_p[:, :].opt()])

            allgather(cc1_in, z1p)

            pools = dict(psum=psum_pool, gather=gpool, s=spool, idx=ipool)

            # ---- hop1: S1 = edges + self; Z2 = dinv^2 * S1 -> cc2_in ----
            hop1_as = dpool.tile([WW, NW, cfg.ZPAD], F16, tag="asm")
            nc.vector.memset(hop1_as[:], 0.0)
            tsum = dpool.tile([WW, C], F32, tag="tsum")

            def drain1(w_, psum_ap):
                nc.vector.tensor_tensor(out=tsum[:], in0=psum_ap,
                                        in1=z0_sb[:, w_, 0:C],
                                        op=mybir.AluOpType.add)
                nc.vector.tensor_scalar_mul(
                    hop1_as[:, w_, 0:C], tsum[:], d2w_sb[:, w_:w_ + 1])

            emit_hop(nc, tc, cfg, sched, pools, z1p, dstw_sb, iota_sb, gidx,
                     drain1, "hop1")
            nc.sync.dma_start(
                out=cc2_in[0:(NW - 1) * WW, :].rearrange(
                    "(w l) c -> l w c", l=WW),
                in_=hop1_as[:, 0:NW - 1, :])
            nc.sync.dma_start(out=cc2_in[(NW - 1) * WW:NPC, :],
                              in_=hop1_as[0:ntail, NW - 1, :])

            allgather(cc2_in, z2p)

            # ---- hop2: S3 = edges + self; logits = dinv*S3 + b ----
            hop2_as = dpool.tile([WW, NW, C], F32, tag="asm2")
            tsum2 = dpool.tile([WW, C], F32, tag="tsum2")

            def drain2(w_, psum_ap):
                nc.vector.tensor_tensor(out=tsum2[:], in0=psum_ap,
                                        in1=hop1_as[:, w_, 0:C],
                                        op=mybir.AluOpType.add)
                nc.vector.tensor_scalar_mul(
                    hop2_as[:, w_, :], tsum2[:], d1w_sb[:, w_:w_ + 1])
                nc.vector.tensor_add(
                    out=hop2_as[:, w_, :], in0=hop2_as[:, w_, :], in1=b_sb[:])

            emit_hop(nc, tc, cfg, sched, pools, z2p, dstw_sb, iota_sb, gidx,
                     drain2, "hop2")

            # ---- log_softmax over C (free axis) ----
            mx = dpool.tile([WW, NW], F32, tag="mx")
            nc.vector.tensor_reduce(out=mx[:], in_=hop2_as[:],
                                    axis=mybir.AxisListType.X,
                                    op=mybir.AluOpType.max)
            tshift = dpool.tile([WW, NW, C], F32, tag="tshift")
            nc.vector.tensor_tensor(
                out=tshift[:], in0=hop2_as[:],
                in1=mx[:].unsqueeze(2).to_broadcast([WW, NW, C]),
                op=mybir.AluOpType.subtract)
            ex = dpool.tile([WW, NW, C], F32, tag="ex")
            nc.scalar.activation(out=ex[:], in_=tshift[:],
                                 func=mybir.ActivationFunctionType.Exp)
            sm = dpool.tile([WW, NW], F32, tag="sm")
            nc.vector.tensor_reduce(out=sm[:], in_=ex[:],
                                    axis=mybir.AxisListType.X,
                                    op=mybir.AluOpType.add)
            lsm = dpool.tile([WW, NW], F32, tag="lsm")
            nc.scalar.activation(out=lsm[:], in_=sm[:],
                                 func=mybir.ActivationFunctionType.Ln)
            res = dpool.tile([WW, NW, C], F32, tag="res")
            nc.vector.tensor_tensor(
                out=res[:], in0=tshift[:],
                in1=lsm[:].unsqueeze(2).to_broadcast([WW, NW, C]),
                op=mybir.AluOpType.subtract)
            nc.sync.dma_start(
                out=out[0:(NW - 1) * WW, :].rearrange("(w l) c -> l w c", l=WW),
                in_=res[:, 0:NW - 1, :])
            nc.sync.dma_start(out=out[(NW - 1) * WW:NPC, :],
                              in_=res[0:ntail, NW - 1, :])
    nc.compile()
    return nc


def kernel(x, edge_index, W, b, _cfg=None, _trace=False, _sim=False):
    global LAST_RESULTS
    cfg = _cfg or Cfg()
    x = np.asarray(x, dtype=np.float32)
    W_ = np.asarray(W, dtype=np.float32)
    b_ = np.asarray(b, dtype=np.float32).reshape(1, cfg.C)
    sched, per_core = preprocess(np.asarray(edge_index), cfg)
    nc = build_program(cfg, sched)

    in_maps = []
    for c in range(cfg.CORES):
        pc = per_core[c]
        in_maps.append({
            "xT": np.ascontiguousarray(x[c * cfg.NPC:(c + 1) * cfg.NPC, :].T),
            "W": W_, "bvec": b_,
            "d1w": pc["d1w"], "d2w": pc["d2w"],
            "gidx": pc["gidx"], "dstw": pc["dstw"],
        })

    if _sim:
        import concourse.bass_interp as bass_interp
        sim = bass_interp.MultiCoreSim(nc, cfg.CORES)
        for c in range(cfg.CORES):
            for k, v in in_maps[c].items():
                sim.cores[c].tensor(k)[:] = v
        sim.simulate()
        outs = [np.array(sim.cores[c].mem_tensor("out"))
                for c in range(cfg.CORES)]
        return np.concatenate(outs, axis=0)

    if _trace:
        import ntff_shim  # noqa: F401
    res = run_bass_kernel_spmd(nc, in_maps, core_ids=list(range(cfg.CORES)),
                               trace=_trace)
    LAST_RESULTS = res
    return np.concatenate([res.results[c]["out"] for c in range(cfg.CORES)],
                          axis=0)
